# revision 3
# baseline (speedup 1.0000x reference)
"""CGC (multi-task MoE) layer on 8 Trainium2 NeuronCores — v2.

Data-parallel over batch (1024 rows/core), weights replicated, zero
collectives.  v2 redesign vs v1:

- LayerNorm mean comes free from the gate matmul: gw is augmented with
  -mean_H(W1_e) columns, so -mu_e per row pops out of PSUM with the
  gate logits.  This removes bn_stats/bn_aggr from the DVE and, more
  importantly, decouples the PSUM->hn relu from the stats chain: the
  relu needs only (-mu), known before L1 even runs.
- Variance via one DVE scalar_tensor_tensor: sum((h-mu)*h) = H*var,
  using the accum_out port; 1/std feeds the *gates* (relu(rs*(h-mu))
  == rs*relu(h-mu) for rs>0), so normalization rides the existing
  per-row gate scalars (grs = softmax_prob * rs) instead of the hot
  PSUM->SBUF activation pass.
- hn transpose batched: one dma_start_transpose per expert (not per
  row-tile) -> 12 SWDGE generations instead of 96.
- L1 PSUM tile is a single 2-bank [128,1024] tile; relu+var each read
  it once, full-width.

Matmuls in bf16 (fp32 PSUM).  Host prep: weight cast/packing, x
transposition, LN-gain folding into W2 (valid when beta==0, gain>=0,
checked at runtime; else the v1 bn_stats path applies gain/beta on
device).
"""

import numpy as np
import ml_dtypes

import concourse.bacc as bacc
import concourse.bass as bass
import concourse.tile as tile
from concourse import mybir
from concourse.bass_utils import run_bass_kernel_spmd

# Problem dims (hardcoded per contest contract).
B, D, H, O = 8192, 512, 1024, 512
T, NE, NS = 2, 4, 4
NEXP = T * NE + NS  # 12
NCORES = 8
EPS = 1e-5
P = 128

FP32 = mybir.dt.float32
BF16 = mybir.dt.bfloat16
AF = mybir.ActivationFunctionType
ALU = mybir.AluOpType

_BF16_NP = ml_dtypes.bfloat16


def _mix_list(e):
    """(acc_key, stream_idx, gate_col) triples for expert e."""
    if e < 4:
        return [("t0", 0, e), ("s", 2, e)]
    if e < 8:
        return [("t1", 1, e - 4), ("s", 2, e)]
    j = e - 8
    return [("t0", 0, 4 + j), ("t1", 1, 4 + j), ("s", 2, 8 + j)]


def build_core_program(rows=1024, with_b1=False, with_b2=False, with_gb=False,
                       ln_affine=False, n_active=NEXP, skip_mixes=False,
                       skip_transpose=False, skip_ln=False, skip_l2=False,
                       repeat=1):
    """Build the per-core Bass program. Returns nc.

    Device inputs (all per-core):
      xt0T/xt1T/xsT: [P, KD, rows] bf16   (feature-major x, chunked over D;
                                           chunk KD-1 is the bias-ones chunk
                                           when with_b1)
      w1:  [NEXP, P, KD, H]  bf16  (chunk k row p = D index k*128+p)
      w2:  [NEXP, P, KH, O]  bf16  (H-chunked; gain pre-folded on host when
                                    not ln_affine; chunk 8 = b2 row if with_b2)
      gw0/gw1: [P, KD, 12] bf16, gws: [P, KD, 16] bf16
          (cols G.. are -mean_H(W1_e) for the stream's experts; the ones
           chunk row 0 holds [gb ; -mean(b1_e)] when with_b1/with_gb)
      lng/lnb: [NEXP, 1, H] fp32                   (only when ln_affine)
    Device outputs:
      outS/out0/out1: [rows//P, P, O] fp32 (row-tile-major)
    """
    assert rows % P == 0
    NM = rows // P
    KD = 4 + (1 if with_b1 else 0)
    KH = 8 + (1 if with_b2 else 0)
    NH2 = 2  # L1 free-dim slices of 512

    nc = bacc.Bacc()

    x_names = ("xt0T", "xt1T", "xsT")
    x_d = [nc.dram_tensor(n, [P, KD, rows], BF16, kind="ExternalInput")
           for n in x_names]
    w1_d = nc.dram_tensor("w1", [NEXP, P, KD, H], BF16, kind="ExternalInput")
    w2_d = nc.dram_tensor("w2", [NEXP, P, KH, O], BF16, kind="ExternalInput")
    G = [NE + NS, NE + NS, NEXP]  # gate widths per stream (8, 8, 12)
    GA = [g + 4 for g in G]       # + 4 mu columns per stream
    gw_d = [nc.dram_tensor(n, [P, KD, ga], BF16, kind="ExternalInput")
            for n, ga in (("gw0", GA[0]), ("gw1", GA[1]), ("gws", GA[2]))]
    lng_d = lnb_d = None
    if ln_affine:
        lng_d = nc.dram_tensor("lng", [NEXP, 1, H], FP32, kind="ExternalInput")
        lnb_d = nc.dram_tensor("lnb", [NEXP, 1, H], FP32, kind="ExternalInput")

    outS_d = nc.dram_tensor("outS", [NM, P, O], FP32, kind="ExternalOutput")
    out0_d = nc.dram_tensor("out0", [NM, P, O], FP32, kind="ExternalOutput")
    out1_d = nc.dram_tensor("out1", [NM, P, O], FP32, kind="ExternalOutput")
    out_d = {"s": outS_d, "t0": out0_d, "t1": out1_d}

    with tile.TileContext(nc) as tc:
        with (
            tc.tile_pool(name="sing", bufs=1) as sing,
            tc.tile_pool(name="wpool", bufs=2) as wpool,
            tc.tile_pool(name="hnp", bufs=2) as hnp,
            tc.tile_pool(name="sqp", bufs=2) as sqp,
            tc.tile_pool(name="stats", bufs=8) as stats,
            tc.tile_pool(name="lnt", bufs=2) as lnt,
            tc.tile_pool(name="ps1", bufs=3, space="PSUM") as ps1,
            tc.tile_pool(name="ps2", bufs=2, space="PSUM") as ps2,
        ):
            # ---- one-time loads: tiny gate weights first, then x ----
            # stream 2 (shared) first: shared experts run first in the
            # expert loop, so its x and gates must land earliest
            STREAM_ORDER = (2, 0, 1)
            gws = [None] * 3
            for i in STREAM_ORDER:
                t = sing.tile([P, KD, GA[i]], BF16, name=f"gw{i}", tag=f"gw{i}")
                nc.sync.dma_start(t[:], gw_d[i][:])
                gws[i] = t
            xs = [None] * 3
            for i in STREAM_ORDER:
                t = sing.tile([P, KD, rows], BF16, name=f"x{i}", tag=f"x{i}")
                for k in range(KD):
                    nc.sync.dma_start(t[:, k, :], x_d[i][:, k, :])
                xs[i] = t
            eps_t = sing.tile([P, 1], FP32)
            nc.vector.memset(eps_t[:], EPS)
            ones_t = None
            if with_b2:
                ones_t = sing.tile([P, rows], BF16)
                nc.vector.memset(ones_t[:], 0.0)
                nc.vector.memset(ones_t[0:1, :], 1.0)

            for _rep in range(repeat):
              # ---- gate phase: softmax(x @ gW) + (-mu) columns ----
              # one PSUM tile per stream (from the ps2 pool, idle during the
              # gate phase): all NM row-tiles' matmuls hit disjoint slices,
              # so the PE never waits on the ACT-side softmax
              gacc, nmus = [None] * 3, [None] * 3
              for s in STREAM_ORDER:
                  gt = sing.tile([P, NM, G[s]], FP32, name=f"gacc{s}",
                                 tag=f"gacc{s}")
                  nm = sing.tile([P, NM, 4], FP32, name=f"nmu{s}",
                                 tag=f"nmu{s}")
                  pg = ps2.tile([P, NM, GA[s]], FP32, name="pg", tag="po")
                  for m in range(NM):
                      for k in range(KD):
                          nc.tensor.matmul(pg[:, m, :],
                                           xs[s][:, k, m * P:(m + 1) * P],
                                           gws[s][:, k, :],
                                           start=(k == 0), stop=(k == KD - 1))
                  # -mu columns straight out of PSUM (one strided copy)
                  nc.scalar.copy(nm[:], pg[:, :, G[s]:])
                  for m in range(NM):
                      # logits are O(1): safe to exp without max-shift;
                      # accum_out gives the softmax denominator for free
                      esum = stats.tile([P, 1], FP32, name="esum", tag="esum")
                      nc.scalar.activation(gt[:, m, :], pg[:, m, :G[s]],
                                           AF.Exp, accum_out=esum[:])
                      rin = stats.tile([P, 1], FP32, name="rin", tag="rin")
                      nc.vector.reciprocal(rin[:], esum[:])
                      nc.vector.tensor_scalar_mul(gt[:, m, :], gt[:, m, :],
                                                  rin[:])
                  gacc[s] = gt
                  nmus[s] = nm

              # per-row gate*rs scalars, filled per expert after its stats
              grs = [sing.tile([P, NM, G[s]], FP32, name=f"grs{s}",
                               tag=f"grs{s}") for s in range(3)]
              if ln_affine:
                  grs = gacc  # rs already folded into hn on-device
              # 1/std per (row, m, expert)
              rss = sing.tile([P, NM, NEXP], FP32, name="rss", tag="rss")

              # ---- output accumulators (row-major fp32) ----
              accs = {k: sing.tile([P, NM, O], FP32, name=f"acc{k}",
                                   tag=f"acc{k}")
                      for k in ("s", "t0", "t1")}
              first = {"s": True, "t0": True, "t1": True}

              # ---- expert loop (pipelined: L2(e-1) emitted after L1(e)) ----
              def do_l1(e):
                  """L1 + LN + relu for expert e -> (hnT, w2t).
                  hnT layout: [P=h%128, m, k=h//128, r] (m-major chunks)."""
                  s = e // 4 if e < 8 else 2
                  w1t = wpool.tile([P, KD, H], BF16, name="w1", tag="w1")
                  nc.sync.dma_start(w1t[:], w1_d[e])
                  w2t = wpool.tile([P, KH, O], BF16, name="w2", tag="w2")
                  nc.sync.dma_start(w2t[:], w2_d[e])
                  g_bc = b_bc = None
                  if ln_affine:
                      g_bc = wpool.tile([P, H], FP32, name="gbc", tag="gbc")
                      nc.sync.dma_start(g_bc[:], lng_d[e].to_broadcast((P, H)))
                      b_bc = wpool.tile([P, H], FP32, name="bbc", tag="bbc")
                      nc.sync.dma_start(b_bc[:], lnb_d[e].to_broadcast((P, H)))

                  hn = hnp.tile([P, NM, H], BF16, name="hn", tag="hn")
                  hnT = hnp.tile([P, NM * (H // P), P], BF16, name="hnT",
                                 tag="hnT")
                  half = NM // 2
                  for m in range(NM):
                      if m == half and not skip_transpose:
                          # first-half transpose while the PE works on the
                          # second half, so L2(e) never waits on the xbar
                          nc.sync.dma_start_transpose(
                              hnT[:, :half * (H // P), :], hn[:, :half, :])
                      ph = ps1.tile([P, H], FP32, name="ph", tag="ph")
                      for k in range(KD):
                          for n in range(NH2):
                              nc.tensor.matmul(
                                  ph[:, n * 512:(n + 1) * 512],
                                  xs[s][:, k, m * P:(m + 1) * P],
                                  w1t[:, k, n * 512:(n + 1) * 512],
                                  start=(k == 0), stop=(k == KD - 1))
                      c = (e - s * 4) if e < 8 else (e - 8)
                      if skip_ln:
                          nc.scalar.activation(hn[:, m, :], ph[:], AF.Relu)
                      elif not ln_affine:
                          nmu = nmus[s][:, m, c:c + 1]
                          # H*var = sum((h-mu)^2) via the ACT accumulator
                          # (walrus rejects DVE accum_out with PSUM source)
                          sq = sqp.tile([P, H], BF16, name="sq", tag="sq")
                          ssq = stats.tile([P, 1], FP32, name="ssq", tag="ssq")
                          nc.scalar.activation(sq[:], ph[:], AF.Square,
                                               bias=nmu, accum_out=ssq[:])
                          # relu on the DVE (tensor_scalar add+max tier),
                          # independent of the stats chain
                          nc.vector.tensor_scalar(
                              hn[:, m, :], ph[:], nmu, 0.0,
                              op0=ALU.add, op1=ALU.max)
                          sd = stats.tile([P, 1], FP32, name="sd", tag="sd")
                          nc.scalar.activation(sd[:], ssq[:], AF.Sqrt,
                                               bias=eps_t[:], scale=1.0 / H)
                          nc.vector.reciprocal(rss[:, m, e:e + 1], sd[:])
                      else:
                          st = stats.tile([P, NH2, 6], FP32, name="bst",
                                          tag="bst")
                          for n in range(NH2):
                              nc.vector.bn_stats(
                                  st[:, n, :], ph[:, n * 512:(n + 1) * 512])
                          mv = stats.tile([P, 2], FP32, name="mv", tag="mv")
                          nc.vector.bn_aggr(mv[:], st[:])
                          std = stats.tile([P, 1], FP32, name="std", tag="std")
                          nc.scalar.activation(std[:], mv[:, 1:2], AF.Sqrt,
                                               bias=eps_t[:])
                          rs = stats.tile([P, 1], FP32, name="rs", tag="rs")
                          nc.vector.reciprocal(rs[:], std[:])
                          nmu2 = stats.tile([P, 1], FP32, name="nmu2",
                                            tag="nmu2")
                          nc.vector.tensor_scalar_mul(nmu2[:], mv[:, 0:1],
                                                      -1.0)
                          tmp = lnt.tile([P, H], FP32, name="lntmp",
                                         tag="lntmp")
                          nc.vector.tensor_scalar(
                              tmp[:], ph[:], nmu2[:], rs[:],
                              op0=ALU.add, op1=ALU.mult)
                          nc.vector.tensor_mul(tmp[:], tmp[:], g_bc[:])
                          nc.vector.tensor_add(tmp[:], tmp[:], b_bc[:])
                          nc.scalar.activation(hn[:, m, :], tmp[:], AF.Relu)

                  if not skip_transpose:
                      nc.sync.dma_start_transpose(
                          hnT[:, half * (H // P):, :], hn[:, half:, :])
                  if not (ln_affine or skip_ln):
                      # grs = softmax_prob * (1/std): one strided [P, NM]
                      # multiply per (mix, expert)
                      for key, si, col in _mix_list(e):
                          nc.vector.tensor_mul(grs[si][:, :, col],
                                               gacc[si][:, :, col],
                                               rss[:, :, e])
                  return hnT, w2t

              def do_l2(e, hnT, w2t, last=False):
                  mixes = _mix_list(e)
                  if skip_mixes:
                      mixes = [mixes[0]]
                  for m in range(NM):
                      po = ps2.tile([P, O], FP32, name="po", tag="po")
                      for k in range(KH):
                          lhs = (hnT[:, m * (H // P) + k, :] if k < 8
                                 else ones_t[:, m * P:(m + 1) * P])
                          nc.tensor.matmul(po[:], lhs, w2t[:, k, :],
                                           start=(k == 0), stop=(k == KH - 1))
                      # gated mixes: acc += po * (gate*rs)
                      for key, si, col in mixes:
                          acc = accs[key][:, m, :]
                          gate_ap = grs[si][:, m, col:col + 1]
                          if first[key]:
                              nc.vector.tensor_scalar_mul(acc, po[:], gate_ap)
                          else:
                              nc.vector.scalar_tensor_tensor(
                                  acc, po[:], gate_ap, acc,
                                  op0=ALU.mult, op1=ALU.add)
                      if last:
                          for key, od in out_d.items():
                              nc.sync.dma_start(od[m], accs[key][:, m, :])
                  for key, _, _ in mixes:
                      first[key] = False

              # shared experts (3 mixes each) first so the tail expert has
              # only 2 mixes draining after the last matmul
              order = ([8, 9, 10, 11] + list(range(8)))[:n_active]
              prev = None
              for e in order:
                  cur = do_l1(e)
                  if prev is not None and not skip_l2:
                      do_l2(*prev)
                  prev = (e, cur[0], cur[1])
              if prev is not None and not skip_l2:
                  do_l2(*prev, last=(n_active == NEXP))

            # ---- store outputs (bulk fallback when not streamed) ----
            if not (n_active == NEXP and not skip_l2):
                for key, od in out_d.items():
                    nc.sync.dma_start(od[:].rearrange("m p f -> p m f"),
                                      accs[key][:])
    nc.finalize()
    return nc


# ---------------- host side ----------------

def _chunk_pf(a, kd):
    """[K*128, F] -> [128, K, F] with row p of chunk k = index k*128+p."""
    k128, f = a.shape
    assert k128 == kd * P
    return np.ascontiguousarray(a.reshape(kd, P, f).transpose(1, 0, 2))


def _pack_xT(x, with_b1):
    """x [rows, D] fp32 -> [P, KD, rows] bf16 (feature-major, chunked)."""
    rows = x.shape[0]
    xT = x.T.astype(_BF16_NP)  # [D, rows]
    out = _chunk_pf(xT, D // P)
    if with_b1:
        aug = np.zeros((P, 1, rows), dtype=_BF16_NP)
        aug[0, 0, :] = 1.0
        out = np.concatenate([out, aug], axis=1)
    return np.ascontiguousarray(out)


def _pack_w1(w1e, b1e, with_b1):
    """W1 [D, H], b1 [H] -> [P, KD, H] bf16."""
    out = _chunk_pf(w1e.astype(_BF16_NP), D // P)
    if with_b1:
        aug = np.zeros((P, 1, H), dtype=_BF16_NP)
        aug[0, 0, :] = b1e.astype(_BF16_NP)
        out = np.concatenate([out, aug], axis=1)
    return np.ascontiguousarray(out)


def _pack_w2(w2e, b2e, ge, fold_g, with_b2):
    """W2 [H, O], b2 [O], g [H] -> [P, KH, O] bf16 (g folded if fold_g)."""
    w = w2e * ge[:, None] if fold_g else w2e
    out = _chunk_pf(w.astype(_BF16_NP), H // P)
    if with_b2:
        aug = np.zeros((P, 1, O), dtype=_BF16_NP)
        aug[0, 0, :] = b2e.astype(_BF16_NP)
        out = np.concatenate([out, aug], axis=1)
    return np.ascontiguousarray(out)


def _pack_gw(gw, gb, w1s, b1s, with_b1, with_gb):
    """gW [D, G], gb [G], W1s [D, 4] (-mean_H W1), b1s [4] (-mean b1)
    -> [P, KD, G+4] bf16."""
    cols = np.concatenate([gw, w1s], axis=1)  # [D, G+4]
    out = _chunk_pf(cols.astype(_BF16_NP), D // P)
    if with_b1:
        aug = np.zeros((P, 1, cols.shape[1]), dtype=_BF16_NP)
        row = np.concatenate([gb if with_gb else np.zeros_like(gb), b1s])
        aug[0, 0, :] = row.astype(_BF16_NP)
        out = np.concatenate([out, aug], axis=1)
    return np.ascontiguousarray(out)


_CACHED = {}


def _get_program(key, **kw):
    if key not in _CACHED:
        _CACHED[key] = build_core_program(**kw)
    return _CACHED[key]


def _prep(inputs):
    """Shared host prep: returns (nc, replicated map, per-core x packer)."""
    f32 = np.float32
    shared_input = np.asarray(inputs["shared_input"], f32)
    task_x = np.asarray(inputs["task_x"], f32)

    # expert order: t0e0..t0e3, t1e0..t1e3, s0..s3
    W1 = np.concatenate([np.asarray(inputs["tW1"], f32).reshape(T * NE, D, H),
                         np.asarray(inputs["sW1"], f32)], axis=0)
    B1 = np.concatenate([np.asarray(inputs["tb1"], f32).reshape(T * NE, H),
                         np.asarray(inputs["sb1"], f32)], axis=0)
    G1 = np.concatenate([np.asarray(inputs["tg"], f32).reshape(T * NE, H),
                         np.asarray(inputs["sg"], f32)], axis=0)
    BT = np.concatenate([np.asarray(inputs["tbeta"], f32).reshape(T * NE, H),
                         np.asarray(inputs["sbeta"], f32)], axis=0)
    W2 = np.concatenate([np.asarray(inputs["tW2"], f32).reshape(T * NE, H, O),
                         np.asarray(inputs["sW2"], f32)], axis=0)
    B2 = np.concatenate([np.asarray(inputs["tb2"], f32).reshape(T * NE, O),
                         np.asarray(inputs["sb2"], f32)], axis=0)
    gW = np.asarray(inputs["gW"], f32)
    gb = np.asarray(inputs["gb"], f32)
    sgW = np.asarray(inputs["sgW"], f32)
    sgb = np.asarray(inputs["sgb"], f32)

    with_b1 = bool(np.any(B1))
    with_b2 = bool(np.any(B2))
    with_gb = bool(np.any(gb)) or bool(np.any(sgb))
    fold_g = bool(np.all(G1 >= 0)) and not np.any(BT)
    ln_affine = not fold_g

    rows = B // NCORES
    nc = _get_program((rows, with_b1, with_b2, with_gb, ln_affine),
                      rows=rows, with_b1=with_b1, with_b2=with_b2,
                      ln_affine=ln_affine)

    w1_np = np.stack([_pack_w1(W1[e], B1[e], with_b1) for e in range(NEXP)])
    w2_np = np.stack([_pack_w2(W2[e], B2[e], G1[e], fold_g, with_b2)
                      for e in range(NEXP)])
    # -mean_H(W1_e) columns per stream (stream experts: 0-3, 4-7, 8-11);
    # mean of the bf16-quantized W1 so mu matches the device-side h
    w1q = W1.astype(_BF16_NP).astype(f32)
    w1s = -w1q.mean(axis=2).T  # [D, NEXP]
    b1s = -B1.mean(axis=1)     # [NEXP]
    gw_np = [
        _pack_gw(gW[0], gb[0], w1s[:, 0:4], b1s[0:4], with_b1, with_gb),
        _pack_gw(gW[1], gb[1], w1s[:, 4:8], b1s[4:8], with_b1, with_gb),
        _pack_gw(sgW, sgb, w1s[:, 8:12], b1s[8:12], with_b1, with_gb),
    ]
    rep = {"w1": w1_np, "w2": w2_np,
           "gw0": gw_np[0], "gw1": gw_np[1], "gws": gw_np[2]}
    if ln_affine:
        rep["lng"] = G1[:, None, :].astype(f32)
        rep["lnb"] = BT[:, None, :].astype(f32)

    in_maps = []
    for c in range(NCORES):
        sl = slice(c * rows, (c + 1) * rows)
        m = dict(rep)
        m["xt0T"] = _pack_xT(task_x[0, sl], with_b1)
        m["xt1T"] = _pack_xT(task_x[1, sl], with_b1)
        m["xsT"] = _pack_xT(shared_input[sl], with_b1)
        in_maps.append(m)
    return nc, in_maps, rows


def kernel(shared_input, task_x, sW1, sb1, sg, sbeta, sW2, sb2,
           tW1, tb1, tg, tbeta, tW2, tb2, gW, gb, sgW, sgb):
    f32 = np.float32
    nc, in_maps, rows = _prep(dict(
        shared_input=shared_input, task_x=task_x, sW1=sW1, sb1=sb1, sg=sg,
        sbeta=sbeta, sW2=sW2, sb2=sb2, tW1=tW1, tb1=tb1, tg=tg, tbeta=tbeta,
        tW2=tW2, tb2=tb2, gW=gW, gb=gb, sgW=sgW, sgb=sgb))

    res = run_bass_kernel_spmd(nc, in_maps, core_ids=list(range(NCORES)))

    outs = {"s": [], "t0": [], "t1": []}
    for c in range(NCORES):
        r = res.results[c]
        outs["s"].append(np.asarray(r["outS"], f32).reshape(rows, O))
        outs["t0"].append(np.asarray(r["out0"], f32).reshape(rows, O))
        outs["t1"].append(np.asarray(r["out1"], f32).reshape(rows, O))
    shared_out = np.concatenate(outs["s"], axis=0)
    t0 = np.concatenate(outs["t0"], axis=0)
    t1 = np.concatenate(outs["t1"], axis=0)
    return (shared_out, t0, t1)


# revision 4
# speedup vs baseline: 2.2374x; 2.2374x over previous
"""CGC (multi-task MoE) layer on 8 Trainium2 NeuronCores — v2.

Data-parallel over batch (1024 rows/core), weights replicated, zero
collectives.  v2 redesign vs v1:

- LayerNorm mean comes free from the gate matmul: gw is augmented with
  -mean_H(W1_e) columns, so -mu_e per row pops out of PSUM with the
  gate logits.  This removes bn_stats/bn_aggr from the DVE and, more
  importantly, decouples the PSUM->hn relu from the stats chain: the
  relu needs only (-mu), known before L1 even runs.
- Variance via one DVE scalar_tensor_tensor: sum((h-mu)*h) = H*var,
  using the accum_out port; 1/std feeds the *gates* (relu(rs*(h-mu))
  == rs*relu(h-mu) for rs>0), so normalization rides the existing
  per-row gate scalars (grs = softmax_prob * rs) instead of the hot
  PSUM->SBUF activation pass.
- hn transpose batched: one dma_start_transpose per expert (not per
  row-tile) -> 12 SWDGE generations instead of 96.
- L1 PSUM tile is a single 2-bank [128,1024] tile; relu+var each read
  it once, full-width.

Matmuls in bf16 (fp32 PSUM).  Host prep: weight cast/packing, x
transposition, LN-gain folding into W2 (valid when beta==0, gain>=0,
checked at runtime; else the v1 bn_stats path applies gain/beta on
device).
"""

import numpy as np
import ml_dtypes

import concourse.bacc as bacc
import concourse.bass as bass
import concourse.tile as tile
from concourse import mybir
from concourse.bass_utils import run_bass_kernel_spmd

# Problem dims (hardcoded per contest contract).
B, D, H, O = 8192, 512, 1024, 512
T, NE, NS = 2, 4, 4
NEXP = T * NE + NS  # 12
NCORES = 8
EPS = 1e-5
P = 128

FP32 = mybir.dt.float32
BF16 = mybir.dt.bfloat16
AF = mybir.ActivationFunctionType
ALU = mybir.AluOpType

_BF16_NP = ml_dtypes.bfloat16


def _mix_list(e):
    """(acc_key, stream_idx, gate_col) triples for expert e."""
    if e < 4:
        return [("t0", 0, e), ("s", 2, e)]
    if e < 8:
        return [("t1", 1, e - 4), ("s", 2, e)]
    j = e - 8
    return [("t0", 0, 4 + j), ("t1", 1, 4 + j), ("s", 2, 8 + j)]


def build_core_program(rows=1024, with_b1=False, with_b2=False, with_gb=False,
                       ln_affine=False, n_active=NEXP, skip_mixes=False,
                       skip_transpose=False, skip_ln=False, skip_l2=False,
                       repeat=1, per_m_transpose=False, act_relu=False,
                       bn_var=False):
    """Build the per-core Bass program. Returns nc.

    Device inputs (all per-core):
      xt0T/xt1T/xsT: [P, KD, rows] bf16   (feature-major x, chunked over D;
                                           chunk KD-1 is the bias-ones chunk
                                           when with_b1)
      w1:  [NEXP, P, KD, H]  bf16  (chunk k row p = D index k*128+p)
      w2:  [NEXP, P, KH, O]  bf16  (H-chunked; gain pre-folded on host when
                                    not ln_affine; chunk 8 = b2 row if with_b2)
      gw0/gw1: [P, KD, 12] bf16, gws: [P, KD, 16] bf16
          (cols G.. are -mean_H(W1_e) for the stream's experts; the ones
           chunk row 0 holds [gb ; -mean(b1_e)] when with_b1/with_gb)
      lng/lnb: [NEXP, 1, H] fp32                   (only when ln_affine)
    Device outputs:
      outS/out0/out1: [rows//P, P, O] fp32 (row-tile-major)
    """
    assert rows % P == 0
    NM = rows // P
    KD = 4 + (1 if with_b1 else 0)
    KH = 8 + (1 if with_b2 else 0)
    NH2 = 2  # L1 free-dim slices of 512

    nc = bacc.Bacc()

    x_names = ("xt0T", "xt1T", "xsT")
    x_d = [nc.dram_tensor(n, [P, KD, rows], BF16, kind="ExternalInput")
           for n in x_names]
    w1_d = nc.dram_tensor("w1", [NEXP, P, KD, H], BF16, kind="ExternalInput")
    w2_d = nc.dram_tensor("w2", [NEXP, P, KH, O], BF16, kind="ExternalInput")
    G = [NE + NS, NE + NS, NEXP]  # gate widths per stream (8, 8, 12)
    GA = [g + 4 for g in G]       # + 4 mu columns per stream
    gw_d = [nc.dram_tensor(n, [P, KD, ga], BF16, kind="ExternalInput")
            for n, ga in (("gw0", GA[0]), ("gw1", GA[1]), ("gws", GA[2]))]
    lng_d = lnb_d = None
    if ln_affine:
        lng_d = nc.dram_tensor("lng", [NEXP, 1, H], FP32, kind="ExternalInput")
        lnb_d = nc.dram_tensor("lnb", [NEXP, 1, H], FP32, kind="ExternalInput")

    outS_d = nc.dram_tensor("outS", [NM, P, O], FP32, kind="ExternalOutput")
    out0_d = nc.dram_tensor("out0", [NM, P, O], FP32, kind="ExternalOutput")
    out1_d = nc.dram_tensor("out1", [NM, P, O], FP32, kind="ExternalOutput")
    out_d = {"s": outS_d, "t0": out0_d, "t1": out1_d}

    with tile.TileContext(nc) as tc:
        with (
            tc.tile_pool(name="sing", bufs=1) as sing,
            tc.tile_pool(name="wpool", bufs=3) as wpool,
            tc.tile_pool(name="hnp", bufs=2) as hnp,
            tc.tile_pool(name="sqp", bufs=2) as sqp,
            tc.tile_pool(name="stats", bufs=8) as stats,
            tc.tile_pool(name="lnt", bufs=2) as lnt,
            tc.tile_pool(name="ps1", bufs=3, space="PSUM") as ps1,
            tc.tile_pool(name="ps2", bufs=2, space="PSUM") as ps2,
        ):
            # ---- one-time loads: tiny gate weights first, then x ----
            # stream 2 (shared) first: shared experts run first in the
            # expert loop, so its x and gates must land earliest; the first
            # two experts' weights are prefetched between x2 and x0/x1 so
            # L1 can start as soon as the gate matmuls for stream 2 finish
            STREAM_ORDER = (2, 0, 1)
            order = ([8, 9, 10, 11] + list(range(8)))[:n_active]
            gws = [None] * 3
            for i in STREAM_ORDER:
                t = sing.tile([P, KD, GA[i]], BF16, name=f"gw{i}", tag=f"gw{i}")
                nc.sync.dma_start(t[:], gw_d[i][:])
                gws[i] = t
            xs = [None] * 3
            preloaded_w = {}
            for i in STREAM_ORDER:
                t = sing.tile([P, KD, rows], BF16, name=f"x{i}", tag=f"x{i}")
                for k in range(KD):
                    nc.sync.dma_start(t[:, k, :], x_d[i][:, k, :])
                xs[i] = t
                if i == STREAM_ORDER[0]:
                    for e in order[:2]:
                        w1t = wpool.tile([P, KD, H], BF16, name="w1",
                                         tag="w1")
                        nc.sync.dma_start(w1t[:], w1_d[e])
                        w2t = wpool.tile([P, KH, O], BF16, name="w2",
                                         tag="w2")
                        nc.sync.dma_start(w2t[:], w2_d[e])
                        preloaded_w[e] = (w1t, w2t)
            eps_t = sing.tile([P, 1], FP32)
            nc.vector.memset(eps_t[:], EPS)
            # preload the ACT function tables during the DMA head: expert
            # set (Square/Sqrt) first, gate set (Exp) last so it's resident
            # when the gate phase starts
            pre_t = sing.tile([P, 2], FP32)
            nc.scalar.activation(pre_t[:, 0:1], eps_t[:], AF.Square)
            nc.scalar.activation(pre_t[:, 1:2], eps_t[:], AF.Sqrt)
            nc.scalar.activation(pre_t[:, 0:1], eps_t[:], AF.Exp)
            ones_t = None
            if with_b2:
                ones_t = sing.tile([P, rows], BF16)
                nc.vector.memset(ones_t[:], 0.0)
                nc.vector.memset(ones_t[0:1, :], 1.0)

            for _rep in range(repeat):
              # ---- gate phase: softmax(x @ gW) + (-mu) columns ----
              # one PSUM tile per stream (from the ps2 pool, idle during the
              # gate phase): all NM row-tiles' matmuls hit disjoint slices,
              # so the PE never waits on the ACT-side softmax
              gacc, nmus = [None] * 3, [None] * 3
              for s in STREAM_ORDER:
                  gt = sing.tile([P, NM, G[s]], FP32, name=f"gacc{s}",
                                 tag=f"gacc{s}")
                  nm = sing.tile([P, NM, 4], FP32, name=f"nmu{s}",
                                 tag=f"nmu{s}")
                  pg = ps2.tile([P, NM, GA[s]], FP32, name="pg", tag="po")
                  for m in range(NM):
                      for k in range(KD):
                          nc.tensor.matmul(pg[:, m, :],
                                           xs[s][:, k, m * P:(m + 1) * P],
                                           gws[s][:, k, :],
                                           start=(k == 0), stop=(k == KD - 1))
                  # -mu columns straight out of PSUM (one strided copy on
                  # the DVE — keeping Copy off the ACT queue avoids
                  # act-table thrash between Copy and Exp sets)
                  nc.vector.tensor_copy(nm[:], pg[:, :, G[s]:])
                  for m in range(NM):
                      # logits are O(1): safe to exp without max-shift;
                      # accum_out gives the softmax denominator for free
                      esum = stats.tile([P, 1], FP32, name="esum", tag="esum")
                      nc.scalar.activation(gt[:, m, :], pg[:, m, :G[s]],
                                           AF.Exp, accum_out=esum[:])
                      rin = stats.tile([P, 1], FP32, name="rin", tag="rin")
                      nc.vector.reciprocal(rin[:], esum[:])
                      nc.vector.tensor_scalar_mul(gt[:, m, :], gt[:, m, :],
                                                  rin[:])
                  gacc[s] = gt
                  nmus[s] = nm

              # per-row gate*rs scalars, filled per expert after its stats
              grs = [sing.tile([P, NM, G[s]], FP32, name=f"grs{s}",
                               tag=f"grs{s}") for s in range(3)]
              if ln_affine:
                  grs = gacc  # rs already folded into hn on-device
              # 1/std per (row, m, expert)
              rss = sing.tile([P, NM, NEXP], FP32, name="rss", tag="rss")

              # ---- output accumulators (row-major fp32) ----
              accs = {k: sing.tile([P, NM, O], FP32, name=f"acc{k}",
                                   tag=f"acc{k}")
                      for k in ("s", "t0", "t1")}
              first = {"s": True, "t0": True, "t1": True}

              # ---- expert loop (pipelined: L2(e-1) emitted after L1(e)) ----
              def do_l1(e):
                  """L1 + LN + relu for expert e -> (hnT, w2t).
                  hnT layout: [P=h%128, m, k=h//128, r] (m-major chunks)."""
                  s = e // 4 if e < 8 else 2
                  if e in preloaded_w:
                      w1t, w2t = preloaded_w.pop(e)
                  else:
                      w1t = wpool.tile([P, KD, H], BF16, name="w1", tag="w1")
                      nc.sync.dma_start(w1t[:], w1_d[e])
                      w2t = wpool.tile([P, KH, O], BF16, name="w2", tag="w2")
                      nc.sync.dma_start(w2t[:], w2_d[e])
                  g_bc = b_bc = None
                  if ln_affine:
                      g_bc = wpool.tile([P, H], FP32, name="gbc", tag="gbc")
                      nc.sync.dma_start(g_bc[:], lng_d[e].to_broadcast((P, H)))
                      b_bc = wpool.tile([P, H], FP32, name="bbc", tag="bbc")
                      nc.sync.dma_start(b_bc[:], lnb_d[e].to_broadcast((P, H)))

                  hn = hnp.tile([P, NM, H], BF16, name="hn", tag="hn")
                  hnT = hnp.tile([P, NM * (H // P), P], BF16, name="hnT",
                                 tag="hnT")
                  half = NM // 2
                  for m in range(NM):
                      if (m == half and not skip_transpose
                              and not per_m_transpose):
                          # first-half transpose while the PE works on the
                          # second half, so L2(e) never waits on the xbar
                          nc.sync.dma_start_transpose(
                              hnT[:, :half * (H // P), :], hn[:, :half, :])
                      ph = ps1.tile([P, H], FP32, name="ph", tag="ph")
                      for k in range(KD):
                          for n in range(NH2):
                              nc.tensor.matmul(
                                  ph[:, n * 512:(n + 1) * 512],
                                  xs[s][:, k, m * P:(m + 1) * P],
                                  w1t[:, k, n * 512:(n + 1) * 512],
                                  start=(k == 0), stop=(k == KD - 1))
                      c = (e - s * 4) if e < 8 else (e - 8)
                      if skip_ln:
                          nc.scalar.activation(hn[:, m, :], ph[:], AF.Relu)
                      elif bn_var and not ln_affine:
                          nmu = nmus[s][:, m, c:c + 1]
                          st = stats.tile([P, NH2, 6], FP32, name="bst",
                                          tag="bst")
                          for n in range(NH2):
                              nc.vector.bn_stats(
                                  st[:, n, :], ph[:, n * 512:(n + 1) * 512])
                          mv = stats.tile([P, 2], FP32, name="mv", tag="mv")
                          nc.vector.bn_aggr(mv[:], st[:])
                          nc.scalar.activation(hn[:, m, :], ph[:], AF.Relu,
                                               bias=nmu)
                          sd = stats.tile([P, 1], FP32, name="sd", tag="sd")
                          nc.scalar.activation(sd[:], mv[:, 1:2], AF.Sqrt,
                                               bias=eps_t[:])
                          nc.vector.reciprocal(rss[:, m, e:e + 1], sd[:])
                      elif not ln_affine:
                          nmu = nmus[s][:, m, c:c + 1]
                          # H*var = sum((h-mu)^2) via the ACT accumulator
                          # (walrus rejects DVE accum_out with PSUM source)
                          sq = sqp.tile([P, H], BF16, name="sq", tag="sq")
                          ssq = stats.tile([P, 1], FP32, name="ssq", tag="ssq")
                          nc.scalar.activation(sq[:], ph[:], AF.Square,
                                               bias=nmu, accum_out=ssq[:])
                          # relu independent of the stats chain: on DVE
                          # (tensor_scalar add+max) or on ACT (A/B flag)
                          if act_relu:
                              nc.scalar.activation(hn[:, m, :], ph[:],
                                                   AF.Relu, bias=nmu)
                          else:
                              nc.vector.tensor_scalar(
                                  hn[:, m, :], ph[:], nmu, 0.0,
                                  op0=ALU.add, op1=ALU.max)
                          sd = stats.tile([P, 1], FP32, name="sd", tag="sd")
                          nc.scalar.activation(sd[:], ssq[:], AF.Sqrt,
                                               bias=eps_t[:], scale=1.0 / H)
                          nc.vector.reciprocal(rss[:, m, e:e + 1], sd[:])
                      else:
                          st = stats.tile([P, NH2, 6], FP32, name="bst",
                                          tag="bst")
                          for n in range(NH2):
                              nc.vector.bn_stats(
                                  st[:, n, :], ph[:, n * 512:(n + 1) * 512])
                          mv = stats.tile([P, 2], FP32, name="mv", tag="mv")
                          nc.vector.bn_aggr(mv[:], st[:])
                          std = stats.tile([P, 1], FP32, name="std", tag="std")
                          nc.scalar.activation(std[:], mv[:, 1:2], AF.Sqrt,
                                               bias=eps_t[:])
                          rs = stats.tile([P, 1], FP32, name="rs", tag="rs")
                          nc.vector.reciprocal(rs[:], std[:])
                          nmu2 = stats.tile([P, 1], FP32, name="nmu2",
                                            tag="nmu2")
                          nc.vector.tensor_scalar_mul(nmu2[:], mv[:, 0:1],
                                                      -1.0)
                          tmp = lnt.tile([P, H], FP32, name="lntmp",
                                         tag="lntmp")
                          nc.vector.tensor_scalar(
                              tmp[:], ph[:], nmu2[:], rs[:],
                              op0=ALU.add, op1=ALU.mult)
                          nc.vector.tensor_mul(tmp[:], tmp[:], g_bc[:])
                          nc.vector.tensor_add(tmp[:], tmp[:], b_bc[:])
                          nc.scalar.activation(hn[:, m, :], tmp[:], AF.Relu)
                      if not skip_transpose and per_m_transpose:
                          nc.sync.dma_start_transpose(
                              hnT[:, m * (H // P):(m + 1) * (H // P), :],
                              hn[:, m, :])

                  if not skip_transpose and not per_m_transpose:
                      nc.sync.dma_start_transpose(
                          hnT[:, half * (H // P):, :], hn[:, half:, :])
                  if not (ln_affine or skip_ln):
                      # grs = softmax_prob * (1/std): one strided [P, NM]
                      # multiply per (mix, expert)
                      for key, si, col in _mix_list(e):
                          nc.vector.tensor_mul(grs[si][:, :, col],
                                               gacc[si][:, :, col],
                                               rss[:, :, e])
                  return hnT, w2t

              def do_l2(e, hnT, w2t):
                  mixes = _mix_list(e)
                  if skip_mixes:
                      mixes = [mixes[0]]
                  # stream each accumulator out as soon as its last
                  # contributing expert's mix lands (t0 finishes 4 experts
                  # before the end with the shared-first ordering)
                  done_keys = [key for key, _, _ in mixes
                               if last_for.get(key) == e]
                  for m in range(NM):
                      po = ps2.tile([P, O], FP32, name="po", tag="po")
                      for k in range(KH):
                          lhs = (hnT[:, m * (H // P) + k, :] if k < 8
                                 else ones_t[:, m * P:(m + 1) * P])
                          nc.tensor.matmul(po[:], lhs, w2t[:, k, :],
                                           start=(k == 0), stop=(k == KH - 1))
                      # gated mixes: acc += po * (gate*rs)
                      for key, si, col in mixes:
                          acc = accs[key][:, m, :]
                          gate_ap = grs[si][:, m, col:col + 1]
                          if first[key]:
                              nc.vector.tensor_scalar_mul(acc, po[:], gate_ap)
                          else:
                              nc.vector.scalar_tensor_tensor(
                                  acc, po[:], gate_ap, acc,
                                  op0=ALU.mult, op1=ALU.add)
                      for key in done_keys:
                          nc.sync.dma_start(out_d[key][m], accs[key][:, m, :])
                  for key, _, _ in mixes:
                      first[key] = False

              # shared experts (3 mixes each) first so the tail expert has
              # only 2 mixes draining after the last matmul
              last_for = {}
              if n_active == NEXP and not skip_l2 and not skip_mixes:
                  for e in order:
                      for key, _, _ in _mix_list(e):
                          last_for[key] = e
              prev = None
              for e in order:
                  cur = do_l1(e)
                  if prev is not None and not skip_l2:
                      do_l2(*prev)
                  prev = (e, cur[0], cur[1])
              if prev is not None and not skip_l2:
                  do_l2(*prev)

            # ---- store outputs (bulk fallback when not streamed) ----
            if not (n_active == NEXP and not skip_l2):
                for key, od in out_d.items():
                    nc.sync.dma_start(od[:].rearrange("m p f -> p m f"),
                                      accs[key][:])
    nc.finalize()
    return nc


# ---------------- host side ----------------

def _chunk_pf(a, kd):
    """[K*128, F] -> [128, K, F] with row p of chunk k = index k*128+p."""
    k128, f = a.shape
    assert k128 == kd * P
    return np.ascontiguousarray(a.reshape(kd, P, f).transpose(1, 0, 2))


def _pack_xT(x, with_b1):
    """x [rows, D] fp32 -> [P, KD, rows] bf16 (feature-major, chunked)."""
    rows = x.shape[0]
    xT = x.T.astype(_BF16_NP)  # [D, rows]
    out = _chunk_pf(xT, D // P)
    if with_b1:
        aug = np.zeros((P, 1, rows), dtype=_BF16_NP)
        aug[0, 0, :] = 1.0
        out = np.concatenate([out, aug], axis=1)
    return np.ascontiguousarray(out)


def _pack_w1(w1e, b1e, with_b1):
    """W1 [D, H], b1 [H] -> [P, KD, H] bf16."""
    out = _chunk_pf(w1e.astype(_BF16_NP), D // P)
    if with_b1:
        aug = np.zeros((P, 1, H), dtype=_BF16_NP)
        aug[0, 0, :] = b1e.astype(_BF16_NP)
        out = np.concatenate([out, aug], axis=1)
    return np.ascontiguousarray(out)


def _pack_w2(w2e, b2e, ge, fold_g, with_b2):
    """W2 [H, O], b2 [O], g [H] -> [P, KH, O] bf16 (g folded if fold_g)."""
    w = w2e * ge[:, None] if fold_g else w2e
    out = _chunk_pf(w.astype(_BF16_NP), H // P)
    if with_b2:
        aug = np.zeros((P, 1, O), dtype=_BF16_NP)
        aug[0, 0, :] = b2e.astype(_BF16_NP)
        out = np.concatenate([out, aug], axis=1)
    return np.ascontiguousarray(out)


def _pack_gw(gw, gb, w1s, b1s, with_b1, with_gb):
    """gW [D, G], gb [G], W1s [D, 4] (-mean_H W1), b1s [4] (-mean b1)
    -> [P, KD, G+4] bf16."""
    cols = np.concatenate([gw, w1s], axis=1)  # [D, G+4]
    out = _chunk_pf(cols.astype(_BF16_NP), D // P)
    if with_b1:
        aug = np.zeros((P, 1, cols.shape[1]), dtype=_BF16_NP)
        row = np.concatenate([gb if with_gb else np.zeros_like(gb), b1s])
        aug[0, 0, :] = row.astype(_BF16_NP)
        out = np.concatenate([out, aug], axis=1)
    return np.ascontiguousarray(out)


_CACHED = {}

# default build configuration (HW-tuned)
DEFAULT_FLAGS = {"bn_var": True}


def _get_program(key, **kw):
    if key not in _CACHED:
        _CACHED[key] = build_core_program(**kw)
    return _CACHED[key]


def _prep(inputs):
    """Shared host prep: returns (nc, replicated map, per-core x packer)."""
    f32 = np.float32
    shared_input = np.asarray(inputs["shared_input"], f32)
    task_x = np.asarray(inputs["task_x"], f32)

    # expert order: t0e0..t0e3, t1e0..t1e3, s0..s3
    W1 = np.concatenate([np.asarray(inputs["tW1"], f32).reshape(T * NE, D, H),
                         np.asarray(inputs["sW1"], f32)], axis=0)
    B1 = np.concatenate([np.asarray(inputs["tb1"], f32).reshape(T * NE, H),
                         np.asarray(inputs["sb1"], f32)], axis=0)
    G1 = np.concatenate([np.asarray(inputs["tg"], f32).reshape(T * NE, H),
                         np.asarray(inputs["sg"], f32)], axis=0)
    BT = np.concatenate([np.asarray(inputs["tbeta"], f32).reshape(T * NE, H),
                         np.asarray(inputs["sbeta"], f32)], axis=0)
    W2 = np.concatenate([np.asarray(inputs["tW2"], f32).reshape(T * NE, H, O),
                         np.asarray(inputs["sW2"], f32)], axis=0)
    B2 = np.concatenate([np.asarray(inputs["tb2"], f32).reshape(T * NE, O),
                         np.asarray(inputs["sb2"], f32)], axis=0)
    gW = np.asarray(inputs["gW"], f32)
    gb = np.asarray(inputs["gb"], f32)
    sgW = np.asarray(inputs["sgW"], f32)
    sgb = np.asarray(inputs["sgb"], f32)

    with_b1 = bool(np.any(B1))
    with_b2 = bool(np.any(B2))
    with_gb = bool(np.any(gb)) or bool(np.any(sgb))
    fold_g = bool(np.all(G1 >= 0)) and not np.any(BT)
    ln_affine = not fold_g

    rows = B // NCORES
    nc = _get_program((rows, with_b1, with_b2, with_gb, ln_affine),
                      rows=rows, with_b1=with_b1, with_b2=with_b2,
                      ln_affine=ln_affine, **DEFAULT_FLAGS)

    w1_np = np.stack([_pack_w1(W1[e], B1[e], with_b1) for e in range(NEXP)])
    w2_np = np.stack([_pack_w2(W2[e], B2[e], G1[e], fold_g, with_b2)
                      for e in range(NEXP)])
    # -mean_H(W1_e) columns per stream (stream experts: 0-3, 4-7, 8-11);
    # mean of the bf16-quantized W1 so mu matches the device-side h
    w1q = W1.astype(_BF16_NP).astype(f32)
    w1s = -w1q.mean(axis=2).T  # [D, NEXP]
    b1s = -B1.mean(axis=1)     # [NEXP]
    gw_np = [
        _pack_gw(gW[0], gb[0], w1s[:, 0:4], b1s[0:4], with_b1, with_gb),
        _pack_gw(gW[1], gb[1], w1s[:, 4:8], b1s[4:8], with_b1, with_gb),
        _pack_gw(sgW, sgb, w1s[:, 8:12], b1s[8:12], with_b1, with_gb),
    ]
    rep = {"w1": w1_np, "w2": w2_np,
           "gw0": gw_np[0], "gw1": gw_np[1], "gws": gw_np[2]}
    if ln_affine:
        rep["lng"] = G1[:, None, :].astype(f32)
        rep["lnb"] = BT[:, None, :].astype(f32)

    in_maps = []
    for c in range(NCORES):
        sl = slice(c * rows, (c + 1) * rows)
        m = dict(rep)
        m["xt0T"] = _pack_xT(task_x[0, sl], with_b1)
        m["xt1T"] = _pack_xT(task_x[1, sl], with_b1)
        m["xsT"] = _pack_xT(shared_input[sl], with_b1)
        in_maps.append(m)
    return nc, in_maps, rows


def kernel(shared_input, task_x, sW1, sb1, sg, sbeta, sW2, sb2,
           tW1, tb1, tg, tbeta, tW2, tb2, gW, gb, sgW, sgb):
    f32 = np.float32
    nc, in_maps, rows = _prep(dict(
        shared_input=shared_input, task_x=task_x, sW1=sW1, sb1=sb1, sg=sg,
        sbeta=sbeta, sW2=sW2, sb2=sb2, tW1=tW1, tb1=tb1, tg=tg, tbeta=tbeta,
        tW2=tW2, tb2=tb2, gW=gW, gb=gb, sgW=sgW, sgb=sgb))

    res = run_bass_kernel_spmd(nc, in_maps, core_ids=list(range(NCORES)))

    outs = {"s": [], "t0": [], "t1": []}
    for c in range(NCORES):
        r = res.results[c]
        outs["s"].append(np.asarray(r["outS"], f32).reshape(rows, O))
        outs["t0"].append(np.asarray(r["out0"], f32).reshape(rows, O))
        outs["t1"].append(np.asarray(r["out1"], f32).reshape(rows, O))
    shared_out = np.concatenate(outs["s"], axis=0)
    t0 = np.concatenate(outs["t0"], axis=0)
    t1 = np.concatenate(outs["t1"], axis=0)
    return (shared_out, t0, t1)
